# revision 30
# baseline (speedup 1.0000x reference)
"""Trainium2 Bass kernel for an image-captioning LSTM decoder.

Model (per reference):
  emb = embedding[captions]                      [B, T, E]
  sum_enc = encoder_out.sum(axis=1)              [B, ENC]
  h0 = mean_enc @ W_h0.T + b_h0 ; c0 likewise
  per step t (Tdec = T-1 steps):
    gates = [emb_t, sum_enc] @ W_ih.T + b_ih + h @ W_hh.T + b_hh
    i,f,g,o -> LSTM update; rows with t >= caption_len-1 frozen, preds 0
    preds_t = h_new @ W_fc.T + b_fc  (masked)

Sharding: pure data-parallel over batch: core c owns rows {c, c+8, ...}
(round-robin over the descending-sorted batch for load balance).  All
weights replicated; zero inter-core communication.

Device algorithm per core (8 rows):
  A) row-sums of encoder_out via one-hot matmuls -> sum_enc [8,512],
     transpose -> sumT; h0/c0/xenc(+biases) GEMMs (full fp32).
  B) EG[t] = emb_t @ W_ih[:, :E].T  (+ xenc + b) precomputed for all
     steps as one GEMM over 504 positions (fp32r), spilled to DRAM.
  C) 63 sequential steps: gates = hT.T @ W_hh.T (fp32r) + EG_t; DVE/ACT
     pointwise; h transposed back into HT (the next step's lhsT and the
     fc GEMM's lhsT), masked by per-position activity.
  D) preds = HT.T @ W_fc.T over 20 vocab chunks of 500, streamed from
     HBM, written straight out per chunk.

Numerics: fp32r matmuls measure ~7e-4 relative error (vs 1e-2 for
bf16); the large-magnitude encoder-sum terms (|preact| ~ 6) go through
full fp32 so the recurrence error stays ~1e-4 per step.
"""

import numpy as np
from contextlib import ExitStack

import concourse.bass as bass
import concourse.tile as tile
from concourse import mybir, bacc
from concourse.bass_utils import run_bass_kernel_spmd
from concourse.masks import make_identity

F32 = mybir.dt.float32
F32R = mybir.dt.float32r

NCORES = 8
B, T, V, E, D, ENC, P = 64, 64, 10000, 512, 512, 512, 196
TD = T - 1            # 63 decode steps
R = B // NCORES       # 8 rows per core
POS = TD * R          # 504 positions per core
KD = D // 128         # 4 k-tiles of the hidden dim
VCH = 500             # vocab chunk (psum bank holds 512 fp32)
NV = V // VCH         # 20 chunks
MTS = [128, 128, 128, POS - 384]   # fc position tiles (504 = 3*128 + 120)

_PROG_CACHE = {}


def _build_program(with_bfc: bool):
    nc = bacc.Bacc("TRN2", target_bir_lowering=False, debug=False,
                   num_devices=NCORES)

    def inp(name, shape, dt=F32):
        return nc.dram_tensor(name, shape, dt, kind="ExternalInput").ap()

    enc = inp("enc", [R, P, ENC], F32R)
    embT = inp("embT", [KD, 128, POS], F32R)
    wembT = inp("wembT", [KD, 128, 4 * D], F32R)
    wencT = inp("wencT", [KD, 128, 4 * D], F32R)
    whhT = inp("whhT", [KD, 128, 4 * D], F32R)
    wh0T = inp("wh0T", [KD, 128, D], F32R)
    wc0T = inp("wc0T", [KD, 128, D], F32R)
    wfcT = inp("wfcT", [KD, 128, V], F32R)
    bsum8 = inp("bsum8", [R, 4 * D])
    bh08 = inp("bh08", [R, D])
    bc08 = inp("bc08", [R, D])
    maskB = inp("maskB", [R, TD])
    rowsel = inp("rowsel", [128, R * R], F32R)
    if with_bfc:
        bfc = inp("bfc", [1, V], F32R)
        mask1 = inp("mask1", [1, POS], F32R)
    preds = nc.dram_tensor("preds", [R, TD, V], F32, kind="ExternalOutput").ap()
    preds_trv = preds.rearrange("r t v -> t r v")


    with tile.TileContext(nc) as tc, ExitStack() as ctx:
        const_pool = ctx.enter_context(tc.tile_pool(name="const", bufs=1))
        state_pool = ctx.enter_context(tc.tile_pool(name="state", bufs=1))

        ident8 = const_pool.tile([R, R], F32, name="ident8")
        make_identity(nc, ident8[:])
        ident8r = const_pool.tile([R, R], F32R, name="ident8r")
        nc.vector.tensor_copy(ident8r[:], ident8[:])
        maskb = const_pool.tile([R, TD], F32, name="maskb")
        nc.sync.dma_start(maskb[:], maskB[:])

        ENGS = [nc.sync, nc.gpsimd, nc.scalar, nc.sync]
        whh_sb = [const_pool.tile([128, 4 * D], F32R, name=f"whh{k}")
                  for k in range(KD)]
        for k in range(KD):
            ENGS[k].dma_start(whh_sb[k][:], whhT[k])

        # HT[k]: columns 8s..8s+8 hold (h after step s-1).T rows 128k..128k+128
        HT = [state_pool.tile([128, R * T], F32R, name=f"HT{k}") for k in range(KD)]
        c_sb = state_pool.tile([R, D], F32, name="c_state")
        EG = [state_pool.tile([128, 4 * D], F32R, name=f"EG{m}")
              for m in range(4)]

        # ---- Phase A: encoder row sums, h0/c0, xenc ----
        with tc.tile_pool(name="pha", bufs=1) as pha, \
             tc.tile_pool(name="phas", bufs=2) as phas, \
             tc.tile_pool(name="pha_ps", bufs=1, space="PSUM") as pha_ps, \
             tc.tile_pool(name="phb_ps", bufs=1, space="PSUM") as phb_ps:
            rsel = pha.tile([128, R * R], F32R, name="rsel")
            nc.sync.dma_start(rsel[:], rowsel[:])

            sum_ps = pha_ps.tile([R, ENC], F32, name="sum_ps")
            nmm = 0
            for r in range(R):
                for c2, (p0, pn) in enumerate(((0, 128), (128, P - 128))):
                    et = phas.tile([128, ENC], F32R, name=f"enc{r}_{c2}",
                                  tag="encchunk")
                    ENGS[r % 4].dma_start(et[:pn, :], enc[r, p0:p0 + pn, :])
                    nc.tensor.matmul(sum_ps[:], rsel[:pn, 8 * r:8 * r + R],
                                     et[:pn, :], start=(nmm == 0),
                                     stop=(nmm == 2 * R - 1))
                    nmm += 1
            sum_sb = pha.tile([R, ENC], F32R, name="sum_sb")
            nc.vector.tensor_copy(sum_sb[:], sum_ps[:])

            # transpose sum -> sumT [128, 8] x4 chunks stored as [128, 32]
            sumT = pha.tile([128, KD * R], F32R, name="sumT")
            for k in range(KD):
                tp = phb_ps.tile([128, R], F32R, name=f"sumT_ps{k}", tag="tps")
                nc.tensor.transpose(tp[:], sum_sb[:, 128 * k:128 * (k + 1)],
                                    ident8r[:])
                nc.vector.tensor_copy(sumT[:, R * k:R * (k + 1)], tp[:])

            # h0 / c0 (weights pre-scaled by 1/196 on host)
            for name, wT, b8, dst in (("h0", wh0T, bh08, None),
                                      ("c0", wc0T, bc08, c_sb)):
                ps = phb_ps.tile([R, D], F32, name=f"{name}_ps", tag="h0ps")
                for k in range(KD):
                    wt = phas.tile([128, D], F32R, name=f"w{name}{k}", tag="w0chunk")
                    ENGS[k].dma_start(wt[:], wT[k])
                    nc.tensor.matmul(ps[:], sumT[:, R * k:R * (k + 1)], wt[:],
                                     start=(k == 0), stop=(k == KD - 1))
                bt = pha.tile([R, D], F32, name=f"b{name}", tag="b0chunk")
                nc.sync.dma_start(bt[:], b8[:])
                if name == "h0":
                    h0_sb = pha.tile([R, D], F32R, name="h0_sb")
                    nc.vector.tensor_add(h0_sb[:], ps[:], bt[:])
                    # transpose h0 into HT block 0
                    for k in range(KD):
                        tp = phb_ps.tile([128, R], F32R, name=f"h0T_ps{k}",
                                         tag="tps")
                        nc.tensor.transpose(tp[:],
                                            h0_sb[:, 128 * k:128 * (k + 1)],
                                            ident8r[:])
                        nc.vector.tensor_copy(HT[k][:, 0:R], tp[:])
                else:
                    nc.vector.tensor_add(dst[:], ps[:], bt[:])

            # xenc = sum_enc @ W_ih[:, E:].T  (full fp32) + bsum
            xe_ps = pha_ps.tile([R, 4 * D], F32, name="xe_ps", tag="bigps")
            for k in range(KD):
                wt = phas.tile([128, 4 * D], F32R, name=f"wenc{k}", tag="wencchunk")
                ENGS[k].dma_start(wt[:], wencT[k])
                for n in range(4):
                    nc.tensor.matmul(xe_ps[:, D * n:D * (n + 1)],
                                     sumT[:, R * k:R * (k + 1)],
                                     wt[:, D * n:D * (n + 1)],
                                     start=(k == 0), stop=(k == KD - 1))
            bs = pha.tile([R, 4 * D], F32, name="bsum_sb")
            nc.sync.dma_start(bs[:], bsum8[:])
            xenc = pha.tile([R, 4 * D], F32, name="xenc")
            nc.vector.tensor_add(xenc[:], xe_ps[:], bs[:])
            # replicate to 128 partitions (16 copies of 8 rows)
            xenc16 = state_pool.tile([128, 4 * D], F32, name="xenc16")
            nc.sync.dma_start(xenc16[0:8, :], xenc[:])
            for w in (8, 16, 32, 64):
                nc.sync.dma_start(xenc16[w:2 * w, :], xenc16[0:w, :])

        # ---- Phase B: EG = embT.T @ wembT (+xenc16) -> DRAM ----
        with tc.tile_pool(name="phb", bufs=1) as phb, \
             tc.tile_pool(name="phb2_ps", bufs=2, space="PSUM") as phb2_ps:
            emb_sb = [phb.tile([128, POS], F32R, name=f"emb_sb{k}",
                               tag=f"embsb{k}") for k in range(KD)]
            wemb_sb = [phb.tile([128, 4 * D], F32R, name=f"wemb_sb{k}",
                                tag=f"wembsb{k}") for k in range(KD)]
            for k in range(KD):
                ENGS[k].dma_start(emb_sb[k][:], embT[k])
                ENGS[(k + 2) % 4].dma_start(wemb_sb[k][:], wembT[k])
            for m in range(4):
                mw = MTS[m]
                eg_ps = phb2_ps.tile([128, 4 * D], F32, name=f"eg_ps{m}",
                                     tag="egps")
                for n in range(4):
                    for k in range(KD):
                        nc.tensor.matmul(
                            eg_ps[:mw, D * n:D * (n + 1)],
                            emb_sb[k][:, 128 * m:128 * m + mw],
                            wemb_sb[k][:, D * n:D * (n + 1)],
                            start=(k == 0), stop=(k == KD - 1))
                nc.vector.tensor_add(EG[m][:mw, :], eg_ps[:mw, :],
                                     xenc16[:mw, :])

        # ---- Phase C: 63 recurrence steps, fc GEMM interleaved ----
        SIG = mybir.ActivationFunctionType.Sigmoid
        TANH = mybir.ActivationFunctionType.Tanh

        with tc.tile_pool(name="phc", bufs=4) as phc, \
             tc.tile_pool(name="phc2", bufs=2) as phc2, \
             tc.tile_pool(name="phd", bufs=4) as phd, \
             tc.tile_pool(name="phd_out", bufs=3) as phd_out, \
             tc.tile_pool(name="gps", bufs=1, space="PSUM") as gps_pool, \
             tc.tile_pool(name="tps", bufs=1, space="PSUM") as tps_pool, \
             tc.tile_pool(name="fps", bufs=3, space="PSUM") as fps_pool:

            if with_bfc:
                bfc_sb = phc.tile([1, V], F32R, name="bfc_sb", tag="bfcsb")
                nc.sync.dma_start(bfc_sb[:], bfc[:])
                m1_sb = phc.tile([1, POS], F32R, name="m1_sb", tag="m1sb")
                nc.sync.dma_start(m1_sb[:], mask1[:])

            def fc_chunk(m, n):
                """preds[postile m, vocab chunk n] = HT.T @ wfc chunk."""
                mw = MTS[m]
                wf = []
                for k in range(KD):
                    wt = phd.tile([128, VCH], F32R, name=f"wf{n}_{m}_{k}",
                                  tag=f"wf{k}")
                    ENGS[k].dma_start(wt[:], wfcT[k][:, VCH * n:VCH * (n + 1)])
                    wf.append(wt)
                ps = fps_pool.tile([128, VCH], F32, name=f"fc{n}_{m}",
                                   tag="fcps")
                for k in range(KD):
                    nc.tensor.matmul(
                        ps[:mw, :],
                        HT[k][:, R + 128 * m:R + 128 * m + mw],
                        wf[k][:], start=(k == 0),
                        stop=(k == KD - 1 and not with_bfc))
                if with_bfc:
                    nc.tensor.matmul(
                        ps[:mw, :], m1_sb[:, 128 * m:128 * m + mw],
                        bfc_sb[:, VCH * n:VCH * (n + 1)],
                        start=False, stop=True)
                ot = phd_out.tile([128, VCH], F32, name=f"fo{n}_{m}",
                                  tag="fcout")
                if n % 2 == 0:
                    nc.vector.tensor_copy(ot[:mw, :], ps[:mw, :])
                else:
                    nc.scalar.activation(
                        ot[:mw, :], ps[:mw, :],
                        mybir.ActivationFunctionType.Copy)
                t0 = 16 * m
                tn = mw // R
                nc.gpsimd.dma_start(
                    preds_trv[t0:t0 + tn, :, VCH * n:VCH * (n + 1)],
                    ot[:mw, :])

            for t in range(TD):
                eg_t = phc.tile([R, 4 * D], F32R, name=f"eg_t{t}", tag="egt")
                j = t % 16
                nc.sync.dma_start(eg_t[:], EG[t // 16][8 * j:8 * (j + 1), :])

                # gate chunk order: [g, i, f, o] (host permutes weight
                # rows).  g,i,f are emitted first so the c-path (pre-f ->
                # sig f -> t2 -> c -> tanh c) runs while the PE chews the
                # fc chunks; the o-bank comes after the fc chunks so only
                # sig(o) -> h -> transpose remains past the PE stream.
                banks = []
                for n in range(3):
                    bk = gps_pool.tile([R, D], F32, name=f"g{t}_{n}",
                                       tag=f"gb{n}")
                    fold = (n == 2)   # f-bank: EG folded so sig(f) reads
                    for k in range(KD):   # PSUM directly (c-path critical)
                        nc.tensor.matmul(
                            bk[:], HT[k][:, R * t:R * (t + 1)],
                            whh_sb[k][:, D * n:D * (n + 1)],
                            start=(k == 0), stop=(k == KD - 1 and not fold))
                    if fold:
                        nc.tensor.matmul(bk[:], ident8r[:],
                                         eg_t[:, D * n:D * (n + 1)],
                                         start=False, stop=True)
                    banks.append(bk)

                m = t // 16 - 1
                if m >= 0:
                    L = min(16 * (m + 2), TD) - 16 * (m + 1)
                    s = t - 16 * (m + 1)
                    for n in range(s * NV // L, (s + 1) * NV // L):
                        fc_chunk(m, n)

                # o-bank: EG folded into the matmul so sigmoid(o) reads
                # PSUM directly (short tail chain)
                bko = gps_pool.tile([R, D], F32, name=f"g{t}_3", tag="gb3")
                for k in range(KD):
                    nc.tensor.matmul(
                        bko[:], HT[k][:, R * t:R * (t + 1)],
                        whh_sb[k][:, 3 * D:4 * D],
                        start=(k == 0), stop=False)
                nc.tensor.matmul(bko[:], ident8r[:], eg_t[:, 3 * D:4 * D],
                                 start=False, stop=True)
                banks.append(bko)

                gs = []
                for n, fn in enumerate((TANH, SIG, SIG)):
                    act = phc2.tile([R, D], F32, name=f"act{t}_{n}",
                                    tag=f"act{n}")
                    if n == 2:
                        nc.scalar.activation(act[:], banks[n][:], fn)
                    else:
                        pre = phc2.tile([R, D], F32, name=f"pre{t}_{n}",
                                        tag=f"pre{n}")
                        nc.vector.tensor_add(pre[:], banks[n][:],
                                             eg_t[:, D * n:D * (n + 1)])
                        nc.scalar.activation(act[:], pre[:], fn)
                    gs.append(act)
                gg, ii, ff = gs
                # sig(o) in halves so the first h chunk unblocks sooner
                oo = phc2.tile([R, D], F32, name=f"act{t}_3", tag="act3")
                for hf in range(2):
                    sl = slice(256 * hf, 256 * (hf + 1))
                    nc.scalar.activation(oo[:, sl], banks[3][:, sl], SIG)
                t1 = phc2.tile([R, D], F32, name=f"t1_{t}", tag="t1")
                nc.vector.tensor_mul(t1[:], ii[:], gg[:])
                t2 = phc2.tile([R, D], F32, name=f"t2_{t}", tag="t2")
                nc.vector.tensor_mul(t2[:], ff[:], c_sb[:])
                nc.vector.tensor_add(c_sb[:], t1[:], t2[:])
                tc_t = phc2.tile([R, D], F32, name=f"tc{t}", tag="tct")
                for hf in range(2):
                    sl = slice(256 * hf, 256 * (hf + 1))
                    nc.scalar.activation(tc_t[:, sl], c_sb[:, sl], TANH)
                # h = (tanh(c) * mask) * sig(o); f32r out, mask folded in
                h_sb = phc2.tile([R, D], F32R, name=f"h{t}", tag="hsb")
                for k in range(KD):
                    sl = slice(128 * k, 128 * (k + 1))
                    nc.vector.scalar_tensor_tensor(
                        h_sb[:, sl], tc_t[:, sl], maskb[:, t:t + 1],
                        oo[:, sl], mybir.AluOpType.mult,
                        mybir.AluOpType.mult)
                # h.T via transpose-mode matmuls into one PSUM tile,
                # then per-k casts into HT (k=0 first so the next step's
                # k=0 gate matmul unblocks earliest)
                tp = tps_pool.tile([128, KD, R], F32R, name=f"hT{t}",
                                   tag="htp")
                for k in range(KD):
                    nc.tensor.transpose(tp[:, k, :],
                                        h_sb[:, 128 * k:128 * (k + 1)],
                                        ident8r[:])
                    nc.vector.tensor_copy(HT[k][:, R * (t + 1):R * (t + 2)],
                                          tp[:, k, :])

            for n in range(20):
                fc_chunk(3, n)

    nc.compile()
    return nc


GPERM = None  # row permutation [g, i, f, o] built lazily


def _gate_perm():
    global GPERM
    if GPERM is None:
        GPERM = np.concatenate([np.arange(2 * D, 3 * D), np.arange(0, D),
                                np.arange(D, 2 * D), np.arange(3 * D, 4 * D)])
    return GPERM


def _chunkT(w):
    """[N, K<=512] weight -> transposed chunks [KD, 128, N] (contiguous)."""
    wt = np.ascontiguousarray(w.T.astype(np.float32))
    return wt.reshape(KD, 128, w.shape[0])


def kernel(encoder_out, encoder_captions, caption_len, embedding,
           W_ih, b_ih, W_hh, b_hh, W_h0, b_h0, W_c0, b_c0, W_fc, b_fc):
    encoder_out = np.asarray(encoder_out, dtype=np.float32)
    encoder_captions = np.asarray(encoder_captions)
    caption_len = np.asarray(caption_len)
    embedding = np.asarray(embedding, dtype=np.float32)
    W_ih = np.asarray(W_ih, dtype=np.float32); b_ih = np.asarray(b_ih, np.float32)
    W_hh = np.asarray(W_hh, dtype=np.float32); b_hh = np.asarray(b_hh, np.float32)
    W_h0 = np.asarray(W_h0, dtype=np.float32); b_h0 = np.asarray(b_h0, np.float32)
    W_c0 = np.asarray(W_c0, dtype=np.float32); b_c0 = np.asarray(b_c0, np.float32)
    W_fc = np.asarray(W_fc, dtype=np.float32); b_fc = np.asarray(b_fc, np.float32)

    with_bfc = bool(np.any(b_fc != 0))
    key = with_bfc
    if key not in _PROG_CACHE:
        _PROG_CACHE[key] = _build_program(with_bfc)
    nc = _PROG_CACHE[key]

    perm = _gate_perm()
    W_ih_p = W_ih[perm]
    W_hh_p = W_hh[perm]
    bsum_p = (b_ih + b_hh)[perm]

    wembT = _chunkT(W_ih_p[:, :E])
    wencT = _chunkT(W_ih_p[:, E:])
    whhT = _chunkT(W_hh_p)
    wh0T = _chunkT(W_h0 / np.float32(P))
    wc0T = _chunkT(W_c0 / np.float32(P))
    wfcT = _chunkT(W_fc)
    bsum8 = np.tile(bsum_p, (R, 1)).astype(np.float32)
    bh08 = np.tile(b_h0, (R, 1)).astype(np.float32)
    bc08 = np.tile(b_c0, (R, 1)).astype(np.float32)
    rowsel = np.zeros((128, R * R), np.float32)
    for r in range(R):
        rowsel[:, 8 * r + r] = 1.0

    in_maps = []
    all_rows = []
    for c in range(NCORES):
        rows = np.arange(c, B, NCORES)
        all_rows.append(rows)
        cap = np.asarray(encoder_captions[rows][:, :TD], dtype=np.int64)
        embg = embedding[cap]                       # [R, TD, E]
        embT = np.ascontiguousarray(
            embg.transpose(2, 1, 0).reshape(E, POS)).reshape(KD, 128, POS)
        dec_len = (caption_len[rows] - 1).astype(np.int64)
        tt = np.arange(TD)[:, None]                 # [TD, 1]
        mpos = (tt < dec_len[None, :]).astype(np.float32).reshape(POS)
        maskB = np.ascontiguousarray(
            (tt < dec_len[None, :]).astype(np.float32).T)   # [R, TD]
        im = dict(enc=np.ascontiguousarray(encoder_out[rows]),
                  embT=embT, wembT=wembT, wencT=wencT, whhT=whhT,
                  wh0T=wh0T, wc0T=wc0T, wfcT=wfcT, bsum8=bsum8,
                  bh08=bh08, bc08=bc08, maskB=maskB, rowsel=rowsel)
        if with_bfc:
            im["bfc"] = b_fc.reshape(1, V).astype(np.float32)
            im["mask1"] = mpos.reshape(1, POS).copy()
        in_maps.append(im)

    global _LAST_IN_MAPS
    _LAST_IN_MAPS = in_maps
    res = run_bass_kernel_spmd(nc, in_maps, list(range(NCORES)))

    out = np.zeros((B, TD, V), np.float32)
    for c in range(NCORES):
        out[all_rows[c]] = res.results[c]["preds"]
    return out



# revision 31
# speedup vs baseline: 1.1914x; 1.1914x over previous
"""Trainium2 Bass kernel for an image-captioning LSTM decoder.

Model (per reference):
  emb = embedding[captions]                      [B, T, E]
  sum_enc = encoder_out.sum(axis=1)              [B, ENC]
  h0 = mean_enc @ W_h0.T + b_h0 ; c0 likewise
  per step t (Tdec = T-1 steps):
    gates = [emb_t, sum_enc] @ W_ih.T + b_ih + h @ W_hh.T + b_hh
    i,f,g,o -> LSTM update; rows with t >= caption_len-1 frozen, preds 0
    preds_t = h_new @ W_fc.T + b_fc  (masked)

Sharding: pure data-parallel over batch: core c owns rows {c, c+8, ...}
(round-robin over the descending-sorted batch for load balance).  All
weights replicated; zero inter-core communication.

Device algorithm per core (8 rows):
  A) row-sums of encoder_out via one-hot matmuls -> sum_enc [8,512],
     transpose -> sumT; h0/c0/xenc(+biases) GEMMs (full fp32).
  B) EG[t] = emb_t @ W_ih[:, :E].T  (+ xenc + b) precomputed for all
     steps as one GEMM over 504 positions (fp32r), spilled to DRAM.
  C) 63 sequential steps: gates = hT.T @ W_hh.T (fp32r) + EG_t; DVE/ACT
     pointwise; h transposed back into HT (the next step's lhsT and the
     fc GEMM's lhsT), masked by per-position activity.
  D) preds = HT.T @ W_fc.T over 20 vocab chunks of 500, streamed from
     HBM, written straight out per chunk.

Numerics: fp32r matmuls measure ~7e-4 relative error (vs 1e-2 for
bf16); the large-magnitude encoder-sum terms (|preact| ~ 6) go through
full fp32 so the recurrence error stays ~1e-4 per step.
"""

import numpy as np
from contextlib import ExitStack

import concourse.bass as bass
import concourse.tile as tile
from concourse import mybir, bacc
from concourse.bass_utils import run_bass_kernel_spmd
from concourse.masks import make_identity

F32 = mybir.dt.float32
F32R = mybir.dt.float32r

NCORES = 8
B, T, V, E, D, ENC, P = 64, 64, 10000, 512, 512, 512, 196
TD = T - 1            # 63 decode steps
R = B // NCORES       # 8 rows per core
POS = TD * R          # 504 positions per core
KD = D // 128         # 4 k-tiles of the hidden dim
VCH = 500             # vocab chunk (psum bank holds 512 fp32)
NV = V // VCH         # 20 chunks
MTS = [128, 128, 128, POS - 384]   # fc position tiles (504 = 3*128 + 120)

_PROG_CACHE = {}


def _build_program(with_bfc: bool):
    nc = bacc.Bacc("TRN2", target_bir_lowering=False, debug=False,
                   num_devices=NCORES)

    def inp(name, shape, dt=F32):
        return nc.dram_tensor(name, shape, dt, kind="ExternalInput").ap()

    enc = inp("enc", [R, P, ENC], F32R)
    embT = inp("embT", [KD, 128, POS], F32R)
    wembT = inp("wembT", [KD, 128, 4 * D], F32R)
    wencT = inp("wencT", [KD, 128, 4 * D], F32R)
    whhT = inp("whhT", [KD, 128, 4 * D], F32R)
    wh0T = inp("wh0T", [KD, 128, D], F32R)
    wc0T = inp("wc0T", [KD, 128, D], F32R)
    wfcT = inp("wfcT", [KD, 128, V], F32R)
    bsum8 = inp("bsum8", [R, 4 * D])
    bh08 = inp("bh08", [R, D])
    bc08 = inp("bc08", [R, D])
    maskB = inp("maskB", [R, TD])
    rowsel = inp("rowsel", [128, R * R], F32R)
    if with_bfc:
        bfc = inp("bfc", [1, V], F32R)
        mask1 = inp("mask1", [1, POS], F32R)
    preds = nc.dram_tensor("preds", [R, TD, V], F32, kind="ExternalOutput").ap()
    preds_trv = preds.rearrange("r t v -> t r v")


    with tile.TileContext(nc) as tc, ExitStack() as ctx:
        const_pool = ctx.enter_context(tc.tile_pool(name="const", bufs=1))
        state_pool = ctx.enter_context(tc.tile_pool(name="state", bufs=1))

        ident8 = const_pool.tile([R, R], F32, name="ident8")
        make_identity(nc, ident8[:])
        ident8r = const_pool.tile([R, R], F32R, name="ident8r")
        nc.vector.tensor_copy(ident8r[:], ident8[:])
        maskb = const_pool.tile([R, TD], F32, name="maskb")
        nc.sync.dma_start(maskb[:], maskB[:])

        ENGS = [nc.sync, nc.gpsimd, nc.scalar, nc.sync]
        whh_sb = [const_pool.tile([128, 4 * D], F32R, name=f"whh{k}")
                  for k in range(KD)]
        for k in range(KD):
            ENGS[k].dma_start(whh_sb[k][:], whhT[k])

        # HT[k]: columns 8s..8s+8 hold (h after step s-1).T rows 128k..128k+128
        HT = [state_pool.tile([128, R * T], F32R, name=f"HT{k}") for k in range(KD)]
        c_sb = state_pool.tile([R, D], F32, name="c_state")
        EG = [state_pool.tile([128, 4 * D], F32R, name=f"EG{m}")
              for m in range(4)]

        # ---- Phase A: encoder row sums, h0/c0, xenc ----
        with tc.tile_pool(name="pha", bufs=1) as pha, \
             tc.tile_pool(name="phas", bufs=2) as phas, \
             tc.tile_pool(name="pha_ps", bufs=1, space="PSUM") as pha_ps, \
             tc.tile_pool(name="phb_ps", bufs=1, space="PSUM") as phb_ps:
            rsel = pha.tile([128, R * R], F32R, name="rsel")
            nc.sync.dma_start(rsel[:], rowsel[:])

            sum_ps = pha_ps.tile([R, ENC], F32, name="sum_ps")
            nmm = 0
            for r in range(R):
                for c2, (p0, pn) in enumerate(((0, 128), (128, P - 128))):
                    et = phas.tile([128, ENC], F32R, name=f"enc{r}_{c2}",
                                  tag="encchunk")
                    ENGS[r % 4].dma_start(et[:pn, :], enc[r, p0:p0 + pn, :])
                    nc.tensor.matmul(sum_ps[:], rsel[:pn, 8 * r:8 * r + R],
                                     et[:pn, :], start=(nmm == 0),
                                     stop=(nmm == 2 * R - 1))
                    nmm += 1
            sum_sb = pha.tile([R, ENC], F32R, name="sum_sb")
            nc.vector.tensor_copy(sum_sb[:], sum_ps[:])

            # transpose sum -> sumT [128, 8] x4 chunks stored as [128, 32]
            sumT = pha.tile([128, KD * R], F32R, name="sumT")
            for k in range(KD):
                tp = phb_ps.tile([128, R], F32R, name=f"sumT_ps{k}", tag="tps")
                nc.tensor.transpose(tp[:], sum_sb[:, 128 * k:128 * (k + 1)],
                                    ident8r[:])
                nc.vector.tensor_copy(sumT[:, R * k:R * (k + 1)], tp[:])

            # h0 / c0 (weights pre-scaled by 1/196 on host)
            for name, wT, b8, dst in (("h0", wh0T, bh08, None),
                                      ("c0", wc0T, bc08, c_sb)):
                ps = phb_ps.tile([R, D], F32, name=f"{name}_ps", tag="h0ps")
                for k in range(KD):
                    wt = phas.tile([128, D], F32R, name=f"w{name}{k}", tag="w0chunk")
                    ENGS[k].dma_start(wt[:], wT[k])
                    nc.tensor.matmul(ps[:], sumT[:, R * k:R * (k + 1)], wt[:],
                                     start=(k == 0), stop=(k == KD - 1))
                bt = pha.tile([R, D], F32, name=f"b{name}", tag="b0chunk")
                nc.sync.dma_start(bt[:], b8[:])
                if name == "h0":
                    h0_sb = pha.tile([R, D], F32R, name="h0_sb")
                    nc.vector.tensor_add(h0_sb[:], ps[:], bt[:])
                    # transpose h0 into HT block 0
                    for k in range(KD):
                        tp = phb_ps.tile([128, R], F32R, name=f"h0T_ps{k}",
                                         tag="tps")
                        nc.tensor.transpose(tp[:],
                                            h0_sb[:, 128 * k:128 * (k + 1)],
                                            ident8r[:])
                        nc.vector.tensor_copy(HT[k][:, 0:R], tp[:])
                else:
                    nc.vector.tensor_add(dst[:], ps[:], bt[:])

            # xenc = sum_enc @ W_ih[:, E:].T  (full fp32) + bsum
            xe_ps = pha_ps.tile([R, 4 * D], F32, name="xe_ps", tag="bigps")
            for k in range(KD):
                wt = phas.tile([128, 4 * D], F32R, name=f"wenc{k}", tag="wencchunk")
                ENGS[k].dma_start(wt[:], wencT[k])
                for n in range(4):
                    nc.tensor.matmul(xe_ps[:, D * n:D * (n + 1)],
                                     sumT[:, R * k:R * (k + 1)],
                                     wt[:, D * n:D * (n + 1)],
                                     start=(k == 0), stop=(k == KD - 1))
            bs = pha.tile([R, 4 * D], F32, name="bsum_sb")
            nc.sync.dma_start(bs[:], bsum8[:])
            xenc = pha.tile([R, 4 * D], F32, name="xenc")
            nc.vector.tensor_add(xenc[:], xe_ps[:], bs[:])
            # replicate to 128 partitions (16 copies of 8 rows)
            xenc16 = state_pool.tile([128, 4 * D], F32, name="xenc16")
            nc.sync.dma_start(xenc16[0:8, :], xenc[:])
            for w in (8, 16, 32, 64):
                nc.sync.dma_start(xenc16[w:2 * w, :], xenc16[0:w, :])

        # ---- Phase B: EG = embT.T @ wembT (+xenc16) -> DRAM ----
        with tc.tile_pool(name="phb", bufs=1) as phb, \
             tc.tile_pool(name="phb2_ps", bufs=2, space="PSUM") as phb2_ps:
            emb_sb = [phb.tile([128, POS], F32R, name=f"emb_sb{k}",
                               tag=f"embsb{k}") for k in range(KD)]
            wemb_sb = [phb.tile([128, 4 * D], F32R, name=f"wemb_sb{k}",
                                tag=f"wembsb{k}") for k in range(KD)]
            for k in range(KD):
                ENGS[k].dma_start(emb_sb[k][:], embT[k])
                ENGS[(k + 2) % 4].dma_start(wemb_sb[k][:], wembT[k])
            for m in range(4):
                mw = MTS[m]
                eg_ps = phb2_ps.tile([128, 4 * D], F32, name=f"eg_ps{m}",
                                     tag="egps")
                for n in range(4):
                    for k in range(KD):
                        nc.tensor.matmul(
                            eg_ps[:mw, D * n:D * (n + 1)],
                            emb_sb[k][:, 128 * m:128 * m + mw],
                            wemb_sb[k][:, D * n:D * (n + 1)],
                            start=(k == 0), stop=(k == KD - 1))
                nc.vector.tensor_add(EG[m][:mw, :], eg_ps[:mw, :],
                                     xenc16[:mw, :])

        # ---- Phase C: 63 recurrence steps, fc GEMM interleaved ----
        SIG = mybir.ActivationFunctionType.Sigmoid
        TANH = mybir.ActivationFunctionType.Tanh

        with tc.tile_pool(name="phc", bufs=4) as phc, \
             tc.tile_pool(name="phc2", bufs=2) as phc2, \
             tc.tile_pool(name="phd", bufs=4) as phd, \
             tc.tile_pool(name="phd_out", bufs=3) as phd_out, \
             tc.tile_pool(name="gps", bufs=1, space="PSUM") as gps_pool, \
             tc.tile_pool(name="tps", bufs=1, space="PSUM") as tps_pool, \
             tc.tile_pool(name="fps", bufs=3, space="PSUM") as fps_pool:

            if with_bfc:
                bfc_sb = phc.tile([1, V], F32R, name="bfc_sb", tag="bfcsb")
                nc.sync.dma_start(bfc_sb[:], bfc[:])
                m1_sb = phc.tile([1, POS], F32R, name="m1_sb", tag="m1sb")
                nc.sync.dma_start(m1_sb[:], mask1[:])

            def fc_chunk(m, n):
                """preds[postile m, vocab chunk n] = HT.T @ wfc chunk."""
                mw = MTS[m]
                wf = []
                for k in range(KD):
                    wt = phd.tile([128, VCH], F32R, name=f"wf{n}_{m}_{k}",
                                  tag=f"wf{k}")
                    ENGS[k].dma_start(wt[:], wfcT[k][:, VCH * n:VCH * (n + 1)])
                    wf.append(wt)
                ps = fps_pool.tile([128, VCH], F32, name=f"fc{n}_{m}",
                                   tag="fcps")
                for k in range(KD):
                    nc.tensor.matmul(
                        ps[:mw, :],
                        HT[k][:, R + 128 * m:R + 128 * m + mw],
                        wf[k][:], start=(k == 0),
                        stop=(k == KD - 1 and not with_bfc))
                if with_bfc:
                    nc.tensor.matmul(
                        ps[:mw, :], m1_sb[:, 128 * m:128 * m + mw],
                        bfc_sb[:, VCH * n:VCH * (n + 1)],
                        start=False, stop=True)
                ot = phd_out.tile([128, VCH], F32, name=f"fo{n}_{m}",
                                  tag="fcout")
                if n % 2 == 0:
                    nc.vector.tensor_copy(ot[:mw, :], ps[:mw, :])
                else:
                    nc.scalar.activation(
                        ot[:mw, :], ps[:mw, :],
                        mybir.ActivationFunctionType.Copy)
                t0 = 16 * m
                tn = mw // R
                nc.gpsimd.dma_start(
                    preds_trv[t0:t0 + tn, :, VCH * n:VCH * (n + 1)],
                    ot[:mw, :])

            for t in range(TD):
                eg_t = phc.tile([R, 4 * D], F32R, name=f"eg_t{t}", tag="egt")
                j = t % 16
                nc.sync.dma_start(eg_t[:], EG[t // 16][8 * j:8 * (j + 1), :])

                # gate chunk order: [g, i, f, o] (host permutes weight
                # rows).  g,i,f are emitted first so the c-path (pre-f ->
                # sig f -> t2 -> c -> tanh c) runs while the PE chews the
                # fc chunks; the o-bank comes after the fc chunks so only
                # sig(o) -> h -> transpose remains past the PE stream.
                banks = []
                for n in range(3):
                    bk = gps_pool.tile([R, D], F32, name=f"g{t}_{n}",
                                       tag=f"gb{n}")
                    for k in range(KD):
                        nc.tensor.matmul(
                            bk[:], HT[k][:, R * t:R * (t + 1)],
                            whh_sb[k][:, D * n:D * (n + 1)],
                            start=(k == 0), stop=(k == KD - 1))
                    banks.append(bk)

                m = t // 16 - 1
                if m >= 0:
                    L = min(16 * (m + 2), TD) - 16 * (m + 1)
                    s = t - 16 * (m + 1)
                    for n in range(s * NV // L, (s + 1) * NV // L):
                        fc_chunk(m, n)

                # o-bank: EG folded into the matmul so sigmoid(o) reads
                # PSUM directly (short tail chain)
                bko = gps_pool.tile([R, D], F32, name=f"g{t}_3", tag="gb3")
                for k in range(KD):
                    nc.tensor.matmul(
                        bko[:], HT[k][:, R * t:R * (t + 1)],
                        whh_sb[k][:, 3 * D:4 * D],
                        start=(k == 0), stop=False)
                nc.tensor.matmul(bko[:], ident8r[:], eg_t[:, 3 * D:4 * D],
                                 start=False, stop=True)
                banks.append(bko)

                gs = []
                for n, fn in enumerate((TANH, SIG, SIG)):
                    act = phc2.tile([R, D], F32, name=f"act{t}_{n}",
                                    tag=f"act{n}")
                    pre = phc2.tile([R, D], F32, name=f"pre{t}_{n}",
                                    tag=f"pre{n}")
                    nc.vector.tensor_add(pre[:], banks[n][:],
                                         eg_t[:, D * n:D * (n + 1)])
                    nc.scalar.activation(act[:], pre[:], fn)
                    gs.append(act)
                gg, ii, ff = gs
                # sig(o) in halves so the first h chunk unblocks sooner
                oo = phc2.tile([R, D], F32, name=f"act{t}_3", tag="act3")
                for hf in range(2):
                    sl = slice(256 * hf, 256 * (hf + 1))
                    nc.scalar.activation(oo[:, sl], banks[3][:, sl], SIG)
                t1 = phc2.tile([R, D], F32, name=f"t1_{t}", tag="t1")
                nc.vector.tensor_mul(t1[:], ii[:], gg[:])
                t2 = phc2.tile([R, D], F32, name=f"t2_{t}", tag="t2")
                nc.vector.tensor_mul(t2[:], ff[:], c_sb[:])
                nc.vector.tensor_add(c_sb[:], t1[:], t2[:])
                tc_t = phc2.tile([R, D], F32, name=f"tc{t}", tag="tct")
                for hf in range(2):
                    sl = slice(256 * hf, 256 * (hf + 1))
                    nc.scalar.activation(tc_t[:, sl], c_sb[:, sl], TANH)
                # h = (tanh(c) * mask) * sig(o); f32r out, mask folded in
                h_sb = phc2.tile([R, D], F32R, name=f"h{t}", tag="hsb")
                for k in range(KD):
                    sl = slice(128 * k, 128 * (k + 1))
                    nc.vector.scalar_tensor_tensor(
                        h_sb[:, sl], tc_t[:, sl], maskb[:, t:t + 1],
                        oo[:, sl], mybir.AluOpType.mult,
                        mybir.AluOpType.mult)
                # h.T via transpose-mode matmuls into one PSUM tile,
                # then per-k casts into HT (k=0 first so the next step's
                # k=0 gate matmul unblocks earliest)
                tp = tps_pool.tile([128, KD, R], F32R, name=f"hT{t}",
                                   tag="htp")
                for k in range(KD):
                    nc.tensor.transpose(tp[:, k, :],
                                        h_sb[:, 128 * k:128 * (k + 1)],
                                        ident8r[:])
                    nc.vector.tensor_copy(HT[k][:, R * (t + 1):R * (t + 2)],
                                          tp[:, k, :])

            for n in range(20):
                fc_chunk(3, n)

    nc.compile()
    return nc


GPERM = None  # row permutation [g, i, f, o] built lazily


def _gate_perm():
    global GPERM
    if GPERM is None:
        GPERM = np.concatenate([np.arange(2 * D, 3 * D), np.arange(0, D),
                                np.arange(D, 2 * D), np.arange(3 * D, 4 * D)])
    return GPERM


def _chunkT(w):
    """[N, K<=512] weight -> transposed chunks [KD, 128, N] (contiguous)."""
    wt = np.ascontiguousarray(w.T.astype(np.float32))
    return wt.reshape(KD, 128, w.shape[0])


def kernel(encoder_out, encoder_captions, caption_len, embedding,
           W_ih, b_ih, W_hh, b_hh, W_h0, b_h0, W_c0, b_c0, W_fc, b_fc):
    encoder_out = np.asarray(encoder_out, dtype=np.float32)
    encoder_captions = np.asarray(encoder_captions)
    caption_len = np.asarray(caption_len)
    embedding = np.asarray(embedding, dtype=np.float32)
    W_ih = np.asarray(W_ih, dtype=np.float32); b_ih = np.asarray(b_ih, np.float32)
    W_hh = np.asarray(W_hh, dtype=np.float32); b_hh = np.asarray(b_hh, np.float32)
    W_h0 = np.asarray(W_h0, dtype=np.float32); b_h0 = np.asarray(b_h0, np.float32)
    W_c0 = np.asarray(W_c0, dtype=np.float32); b_c0 = np.asarray(b_c0, np.float32)
    W_fc = np.asarray(W_fc, dtype=np.float32); b_fc = np.asarray(b_fc, np.float32)

    with_bfc = bool(np.any(b_fc != 0))
    key = with_bfc
    if key not in _PROG_CACHE:
        _PROG_CACHE[key] = _build_program(with_bfc)
    nc = _PROG_CACHE[key]

    perm = _gate_perm()
    W_ih_p = W_ih[perm]
    W_hh_p = W_hh[perm]
    bsum_p = (b_ih + b_hh)[perm]

    wembT = _chunkT(W_ih_p[:, :E])
    wencT = _chunkT(W_ih_p[:, E:])
    whhT = _chunkT(W_hh_p)
    wh0T = _chunkT(W_h0 / np.float32(P))
    wc0T = _chunkT(W_c0 / np.float32(P))
    wfcT = _chunkT(W_fc)
    bsum8 = np.tile(bsum_p, (R, 1)).astype(np.float32)
    bh08 = np.tile(b_h0, (R, 1)).astype(np.float32)
    bc08 = np.tile(b_c0, (R, 1)).astype(np.float32)
    rowsel = np.zeros((128, R * R), np.float32)
    for r in range(R):
        rowsel[:, 8 * r + r] = 1.0

    in_maps = []
    all_rows = []
    for c in range(NCORES):
        rows = np.arange(c, B, NCORES)
        all_rows.append(rows)
        cap = np.asarray(encoder_captions[rows][:, :TD], dtype=np.int64)
        embg = embedding[cap]                       # [R, TD, E]
        embT = np.ascontiguousarray(
            embg.transpose(2, 1, 0).reshape(E, POS)).reshape(KD, 128, POS)
        dec_len = (caption_len[rows] - 1).astype(np.int64)
        tt = np.arange(TD)[:, None]                 # [TD, 1]
        mpos = (tt < dec_len[None, :]).astype(np.float32).reshape(POS)
        maskB = np.ascontiguousarray(
            (tt < dec_len[None, :]).astype(np.float32).T)   # [R, TD]
        im = dict(enc=np.ascontiguousarray(encoder_out[rows]),
                  embT=embT, wembT=wembT, wencT=wencT, whhT=whhT,
                  wh0T=wh0T, wc0T=wc0T, wfcT=wfcT, bsum8=bsum8,
                  bh08=bh08, bc08=bc08, maskB=maskB, rowsel=rowsel)
        if with_bfc:
            im["bfc"] = b_fc.reshape(1, V).astype(np.float32)
            im["mask1"] = mpos.reshape(1, POS).copy()
        in_maps.append(im)

    global _LAST_IN_MAPS
    _LAST_IN_MAPS = in_maps
    res = run_bass_kernel_spmd(nc, in_maps, list(range(NCORES)))

    out = np.zeros((B, TD, V), np.float32)
    for c in range(NCORES):
        out[all_rows[c]] = res.results[c]["preds"]
    return out

